# revision 21
# baseline (speedup 1.0000x reference)
"""Conv2d(32->32, 3x3, stride 1, pad 1) on X[32,32,224,224] fp32, data-parallel
over 8 NeuronCores (4 images per core).

Per-core algorithm ("row-rotated, gpsimd-rotation, col-tiled pairs")
--------------------------------------------------------------------
The conv is computed as PE matmuls with contraction K = 128 = (q in 0..3
row-taps) x (c = 32 input channels), N = 448 = (u in 0..1 row-pairs) x
(w in 0..223), in fp16.  Each matmul has M = 64 = (ho in 0..1) x (k = 32
output channels); TWO such matmuls are packed per PSUM tile via PE column
tiling (tile_position (0,0)/(0,64) auto-derived from the PSUM base
partition), so the array runs at effective M = 128.

X (host-padded to 226 wide / 228 rows, fp16, stored ONCE) is DMA'd into the
row-rotated SBUF layout xr0[32*q + c, jd, w] holding padded row j = 4*jd+q.
The odd-parity copy xr1 (same layout rotated by two rows) is built by two
GpSimd SBUF->SBUF tensor_copies per slice; the two copies touch disjoint
Q7-core halves and run concurrently, keeping the DMA engines free for pure
HBM traffic.  Per 8-row group i: pair A (rows 8i+{0,1,4,5}, G = h%4 in
{0,1}, rhs xr0) streams into PE cols 0..63 and pair B (rows 8i+{2,3,6,7},
G in {2,3}, rhs xr1) into cols 64..127, each a 3-matmul PSUM accumulation
over the column shift s.

Evictions are full-width [128, 2, 224] PSUM->SBUF ops (ScalarE/VectorE
alternating) with fused bias, casting to fp16 staging ysb[32G + k, m, w]
(output row h = 4m + G).  One store DMA per (image, 112-row H-slice) writes
the G-major DRAM layout Yg[n, G, k, m, w] with 12.5 KB contiguous
per-partition descriptors; the host transposes (n,G,k,m,w) -> (n,k,4m+G,w)
and upcasts to fp32.
"""

import sys

import numpy as np

try:
    import concourse.bass as bass  # noqa: F401
except ImportError:  # pragma: no cover
    sys.path.insert(0, "/opt/trn_rl_repo")

import ml_dtypes
import concourse.mybir as mybir
import concourse.tile as tile
from concourse import bacc
from concourse.bass_utils import run_bass_kernel_spmd

NCORES = 8
NB = 4  # images per core
C = 32
K = 32
H = 224
W = 224
WP = 226  # padded width
NQ = 57  # row-quads in the host-rotated layout (228 padded rows / 4)
RS = 112  # output rows per slice
NSLICE = H // RS
NJD = RS // 4 + 1  # row-quads per rotated slice tile (29)
NG = RS // 8  # 8-row groups per slice (14)
F32 = mybir.dt.float32
F16 = mybir.dt.float16
AF = mybir.ActivationFunctionType
_NP16 = np.float16


def set_dtype(name):
    """'fp16' (default) or 'bf16' for the matmul operand precision."""
    global F16, _NP16, _NC
    if name == "bf16":
        F16, _NP16 = mybir.dt.bfloat16, ml_dtypes.bfloat16
    else:
        F16, _NP16 = mybir.dt.float16, np.float16
    _NC = None


def conv_body(tc, X, Wt, Bias, Y):
    nc = tc.nc
    with (
        tc.tile_pool(name="const", bufs=1) as cpool,
        tc.tile_pool(name="xpool", bufs=3) as xpool,
        tc.tile_pool(name="ypool", bufs=3) as ypool,
        tc.tile_pool(name="ppool", bufs=8, space="PSUM") as ppool,
    ):
        # weights/bias on the scalar HWDGE ring so the first X load (sync
        # ring) is not queued behind their ~2us fixed latencies
        wt_sb = cpool.tile([128, 3, 64], F16)
        nc.scalar.dma_start(out=wt_sb[:], in_=Wt)
        b_sb = cpool.tile([128, 1], F32)
        nc.scalar.dma_start(out=b_sb[:], in_=Bias)

        # Yg[n, G, k, m, w] = output row h = 4m + G of image n: partition
        # dim (G, k) = 32G + k matches the eviction staging layout, and the
        # per-partition (m, w) block is contiguous in DRAM.
        Yv = [Y[g].rearrange("g k m w -> (g k) m w") for g in range(NB)]

        # PE warm-up: dummy matmuls over the (tiny, already-loaded) weight
        # tile keep the HAM activity window busy through the NEFF preamble +
        # first load, so the first real matmuls run at 2.4 GHz instead of
        # 1.2.  Shares the pt ring tag, so the first real group simply
        # queues behind the last dummy.
        wpt = ppool.tile([128, 2, 224], F32, name="pt", tag="pt")
        for _ in range(24):
            nc.tensor.matmul(
                wpt[0:64, 0, 0:192],
                wt_sb[:, 0, :],
                wt_sb.rearrange("p a b -> p (a b)"),
                start=True,
                stop=True,
            )

        for n in range(NB):
            for t in range(NSLICE):
                # X arrives host-rotated: X[n, q, c, jd, w] = row j = 4*jd+q.
                xr0 = xpool.tile([128, NJD, WP], F16, name="xr0", tag="xr0")
                j0 = (NJD - 1) * t
                if n == 0 and t == 0:
                    # split the first load so the copies + first matmuls
                    # start ~half a load earlier
                    nc.sync.dma_start(
                        out=xr0[:, 0:15, :], in_=X[n, :, :, 0:15, :]
                    )
                    nc.sync.dma_start(
                        out=xr0[:, 15:NJD, :], in_=X[n, :, :, 15:NJD, :]
                    )
                else:
                    nc.sync.dma_start(
                        out=xr0[:, :, :], in_=X[n, :, :, j0 : j0 + NJD, :]
                    )
                # xr1[32q'+c, jq, w] holds local row 4*jq + q' + 2:
                #   q' in {2,3}: = xr0's (q=q'-2, jd=jq+1)
                #   q' in {0,1}: = xr0's (q=q'+2, jd=jq)
                # Two DVE fp16 copies (2x perf mode) on partition halves;
                # DMA engines stay on pure HBM work.  First slice: copies
                # split to chase the two load halves.
                xr1 = xpool.tile([128, NJD, WP], F16, name="xr1", tag="xr1")
                if n == 0 and t == 0:
                    nc.vector.tensor_copy(
                        xr1[64:128, 0:14, :], xr0[0:64, 1:15, :]
                    )
                    nc.vector.tensor_copy(
                        xr1[0:64, 0:14, :], xr0[64:128, 0:14, :]
                    )
                    nc.vector.tensor_copy(
                        xr1[64:128, 14 : NJD - 1, :], xr0[0:64, 15:NJD, :]
                    )
                    nc.vector.tensor_copy(
                        xr1[0:64, 14 : NJD - 1, :],
                        xr0[64:128, 14 : NJD - 1, :],
                    )
                else:
                    nc.vector.tensor_copy(
                        xr1[64:128, 0 : NJD - 1, :], xr0[0:64, 1:NJD, :]
                    )
                    nc.vector.tensor_copy(
                        xr1[0:64, 0 : NJD - 1, :], xr0[64:128, 0 : NJD - 1, :]
                    )

                # fp16 staging for the slice: partition (G = h%4, k),
                # free (m = h//4 local, w)
                ysb = ypool.tile([128, RS // 4, 224], F16, name="ysb", tag="ysb")
                for i in range(NG):  # 8-row group 8i..8i+7
                    pt = ppool.tile([128, 2, 224], F32, name="pt", tag="pt")
                    # pairs A (G in {0,1}, rows 8i+4u+ho, cols 0..63) and B
                    # (G in {2,3}, rows +2, cols 64..127), INTERLEAVED per
                    # s so neither column half waits out the other's burst
                    # (matmul starts are pc-monotone)
                    for s in range(3):
                        nc.tensor.matmul(
                            pt[0:64, :, :],
                            wt_sb[:, s, :],
                            xr0[:, 2 * i : 2 * i + 2, s : s + 224],
                            start=(s == 0),
                            stop=(s == 2),
                            skip_group_check=True,
                        )
                        nc.tensor.matmul(
                            pt[64:128, :, :],
                            wt_sb[:, s, :],
                            xr1[:, 2 * i : 2 * i + 2, s : s + 224],
                            start=(s == 0),
                            stop=(s == 2),
                            skip_group_check=True,
                        )
                    # full-width eviction: partitions already (G, k); free
                    # offset m = 2i + u for both halves.  ScalarE everywhere
                    # (DVE must keep the rotation copies snappy), except the
                    # last slice where DVE is free to help drain the tail.
                    dst = ysb[:, 2 * i : 2 * i + 2, :]
                    last = n == NB - 1 and t == NSLICE - 1
                    if last and i % 2 == 1:
                        nc.vector.tensor_scalar_add(dst, pt[:, :, :], b_sb[:, :])
                    else:
                        nc.scalar.activation(
                            dst, pt[:, :, :], AF.Identity, bias=b_sb[:, :]
                        )
                m0 = RS // 4 * t
                if n == NB - 1 and t == NSLICE - 1:
                    # quarter the last store so the final drain is short
                    for m in range(0, 28, 7):
                        nc.sync.dma_start(
                            out=Yv[n][:, m0 + m : m0 + m + 7, :],
                            in_=ysb[:, m : m + 7, :],
                        )
                else:
                    nc.scalar.dma_start(
                        out=Yv[n][:, m0 : m0 + RS // 4, :], in_=ysb[:, :, :]
                    )


def build_nc(nb=NB, repeat=1):
    assert nb == NB
    nc = bacc.Bacc("TRN2", target_bir_lowering=False, debug=False)
    X = nc.dram_tensor("X", [NB, 4, C, NQ, WP], F16, kind="ExternalInput").ap()
    Wt = nc.dram_tensor("Wt", [128, 3, 64], F16, kind="ExternalInput").ap()
    Bias = nc.dram_tensor("bias", [128, 1], F32, kind="ExternalInput").ap()
    Y = nc.dram_tensor(
        "Y", [NB, 4, K, H // 4, W], F16, kind="ExternalOutput"
    ).ap()
    with tile.TileContext(nc) as tc:
        if repeat == 1:
            conv_body(tc, X, Wt, Bias, Y)
        else:
            with tc.For_i(0, repeat, 1):
                conv_body(tc, X, Wt, Bias, Y)
    nc.compile()
    return nc


def prep_weights(Wf, b):
    """Wt[32*q+c, s, 32*ho+k] = W[k, c, q-ho, s] (0 outside 0<=r<3)."""
    Wf = np.asarray(Wf, np.float32)
    Wt = np.zeros((128, 3, 64), np.float32)
    for q in range(4):
        for ho in range(2):
            r = q - ho
            if 0 <= r <= 2:
                Wt[32 * q : 32 * q + 32, :, 32 * ho : 32 * ho + 32] = Wf[
                    :, :, r, :
                ].transpose(1, 2, 0)
    bias = np.tile(np.asarray(b, np.float32), 4).reshape(128, 1)
    return Wt.astype(_NP16), bias


def pad_input(X):
    """Pad to 228x226 and pre-rotate rows: out[n, q, c, jd, w] = row 4*jd+q."""
    X = np.ascontiguousarray(X, np.float32)
    Xp = np.zeros((X.shape[0], C, H + 4, WP), _NP16)
    Xp[:, :, 1 : H + 1, 1 : W + 1] = X
    Xr = Xp.reshape(X.shape[0], C, NQ, 4, WP).transpose(0, 3, 1, 2, 4)
    return np.ascontiguousarray(Xr)


_NC = None


def _get_nc():
    global _NC
    if _NC is None:
        _NC = build_nc(NB)
    return _NC


def kernel(X, W, b, _trace=False):
    Xp = pad_input(X)
    Wt, bias = prep_weights(W, b)
    nc = _get_nc()
    in_maps = [
        {"X": Xp[NB * c : NB * (c + 1)], "Wt": Wt, "bias": bias} for c in range(NCORES)
    ]
    res = run_bass_kernel_spmd(nc, in_maps, list(range(NCORES)), trace=_trace)
    # Yg[n, G, k, m, w] -> Y[n, k, 4m+G, w]
    out = np.concatenate(
        [
            np.ascontiguousarray(
                res.results[c]["Y"].transpose(0, 2, 3, 1, 4)
            ).reshape(NB, 32, 224, 224)
            for c in range(NCORES)
        ],
        axis=0,
    ).astype(np.float32)
    if _trace:
        return out, res
    return out
